# revision 39
# baseline (speedup 1.0000x reference)
"""AttentionWithRotary on 8 Trainium2 cores.

Math: reference applies raw (no-softmax) attention:
    out = ((rope(xWq^T+bq) @ rope(xWk^T+bk)^T)/sqrt(hd)) @ (xWv^T+bv) @ Wo^T + bo
Since there is no softmax, associativity gives per head:
    out_h = Q_r @ (K_r^T @ V) / sqrt(hd)
which turns the S x S score matrix into a hd x hd one.

Sharding: data-parallel on batch (2) x tensor-parallel on heads (4 heads/core).
Each core computes a row-parallel partial of the output projection; partials
are summed on the host (the "all-reduce" of row-parallel TP) and bo added.

Per-core layouts (prepped on host, all bf16):
  xT   [2048,2048]  x[b]^T  (din on partitions)
  wqT/wkT/wvT [2048,512]  weight slices transposed; q/k rows pair-split
                 permuted per head (64 even pair elems then 64 odd); q scaled
                 by 1/sqrt(hd)
  woT  [512,2048]  wo column slice transposed
  c2   [128,2048]  [cosT; cosT]        (cosT = freqs_cos^T, [64,2048])
  s2x  [128,2048]  [sinT; -sinT]
  cn4  [2048,512]  per-head [cos,cos] tiling (natural layout)
  sn4x [2048,512]  per-head [sin,-sin] tiling
  bq/bk/bv [1,512] bias slices (q/k permuted, q scaled)

On-device per core:
  QT (rope) [512,2048] transposed layout; K_r,V natural [2048,512] streamed;
  Mh[h] = K_r^T V accumulated in one PSUM bank across all seq tiles;
  attn^T[h] = Mh[h]^T @ QT[h]; partial = attn @ woT accumulated over heads.
"""
import numpy as np
import ml_dtypes
from contextlib import ExitStack

import concourse.bacc as bacc
import concourse.tile as tile
import concourse.mybir as mybir
from concourse.bass_utils import run_bass_kernel_spmd

BF16 = mybir.dt.bfloat16
F32 = mybir.dt.float32
NPBF = ml_dtypes.bfloat16

S = 2048
DIN = 2048
NH = 16
HD = 128
B = 2
NCORES = 8
TP = 4                 # head-parallel ways
NHL = NH // TP         # 4 heads per core
DLOC = NHL * HD        # 512 local head dims
BLK = 512
NBLK = S // BLK        # 4 seq blocks
KT = DIN // 128        # 16 contraction tiles
P = 128

_NC_CACHE = None


def _emit(nc, tc, ctx, d, out_d, dbg=None, reps=1, loop_n=0):
    wpool = ctx.enter_context(tc.tile_pool(name="w", bufs=1))
    xpool = ctx.enter_context(tc.tile_pool(name="x", bufs=2))
    tpool = ctx.enter_context(tc.tile_pool(name="trig", bufs=3))
    qpool = ctx.enter_context(tc.tile_pool(name="qtr", bufs=1))
    kvpool = ctx.enter_context(tc.tile_pool(name="kv", bufs=3))
    tmp = ctx.enter_context(tc.tile_pool(name="tmp", bufs=3))
    osb = ctx.enter_context(tc.tile_pool(name="osb", bufs=4))
    psum = ctx.enter_context(tc.tile_pool(name="ps", bufs=7, space="PSUM"))
    mps = ctx.enter_context(tc.tile_pool(name="mps", bufs=1, space="PSUM"))

    def load_xt(blk):
        # per-k 2D DMAs (contiguous free dim) into one wide tile; 3D
        # k-batched APs compile but are ~30x slower on real hardware
        t = xpool.tile([P, KT * BLK], BF16, name="xt", tag="xt")
        for k in range(KT):
            nc.gpsimd.dma_start(
                t[:, k * BLK:(k + 1) * BLK],
                d["xT"][k * 128:(k + 1) * 128, blk * BLK:(blk + 1) * BLK])
        return t

    # --- resident constants, in consumption order so the DMA queue serves
    # the first matmuls quickly ---
    ones = wpool.tile([1, BLK], BF16, name="ones", tag="ones")
    nc.vector.memset(ones[:], 1.0)
    bias = {}
    for nm in ("bk", "bv"):
        t = wpool.tile([1, DLOC], BF16, name=nm, tag=nm)
        nc.sync.dma_start(t[:], d[nm][:])
        bias[nm] = t
    bqc = wpool.tile([P, NHL], BF16, name="bqc", tag="bqc")
    nc.sync.dma_start(bqc[:], d["bqc"][:])

    xt0 = load_xt(0)
    wq_all = wpool.tile([P, KT * DLOC], BF16, name="wq_all", tag="wq_all")
    for k in range(KT):
        nc.sync.dma_start(wq_all[:, k * DLOC:(k + 1) * DLOC],
                          d["wqT"][k * 128:(k + 1) * 128, :])
    c2 = wpool.tile([P, S], BF16, name="c2", tag="c2")
    nc.sync.dma_start(c2[:], d["c2"][:])
    s2x = wpool.tile([P, S], BF16, name="s2x", tag="s2x")
    nc.sync.dma_start(s2x[:], d["s2x"][:])
    wk_all = wpool.tile([P, KT * DLOC], BF16, name="wk_all", tag="wk_all")
    wv_all = wpool.tile([P, KT * DLOC], BF16, name="wv_all", tag="wv_all")
    for k in range(KT):
        nc.sync.dma_start(wk_all[:, k * DLOC:(k + 1) * DLOC],
                          d["wkT"][k * 128:(k + 1) * 128, :])
        nc.sync.dma_start(wv_all[:, k * DLOC:(k + 1) * DLOC],
                          d["wvT"][k * 128:(k + 1) * 128, :])
    wo_all = wpool.tile([P, NHL * DIN], BF16, name="wo_all", tag="wo_all")
    for h in range(NHL):
        nc.sync.dma_start(wo_all[:, h * DIN:(h + 1) * DIN],
                          d["woT"][h * 128:(h + 1) * 128, :])

    qtr = []
    for h in range(NHL):
        t = qpool.tile([P, S], BF16, name=f"qtr{h}", tag=f"qtr{h}")
        qtr.append(t)

    def pair_view(t):
        return t.rearrange("p (h u j) -> p h u j", h=NHL, u=2, j=64)

    if loop_n:
        with tc.For_i(0, loop_n, 1) as _i:
            _emit_compute(nc, tc, d, out_d, dbg, tpool, kvpool, tmp,
                          osb, psum, mps, ones, bias, bqc, c2, s2x, wq_all,
                          wk_all, wv_all, wo_all, qtr, pair_view, load_xt,
                          None)
    else:
        for _rep in range(reps):
            _emit_compute(nc, tc, d, out_d, dbg, tpool, kvpool, tmp,
                          osb, psum, mps, ones, bias, bqc, c2, s2x, wq_all,
                          wk_all, wv_all, wo_all, qtr, pair_view, load_xt,
                          xt0 if _rep == 0 else None)


def _emit_compute(nc, tc, d, out_d, dbg, tpool, kvpool, tmp, osb, psum,
                  mps, ones, bias, bqc, c2, s2x, wq_all, wk_all, wv_all,
                  wo_all, qtr, pair_view, load_xt, xt0):
    ADD = mybir.AluOpType.add
    MULT = mybir.AluOpType.mult
    mh_ps = mps.tile([P, DLOC], F32, name="mh_ps", tag="mh")

    def emit_mh(kr, vt, mg):
        if mg == 0:
            # start=True clears the whole PSUM bank, so only the first
            # matmul may carry it; the critical section pins the order
            # of the four first-writes within the shared bank.
            with tc.tile_critical():
                for h in range(NHL):
                    h0, h1 = h * 128, (h + 1) * 128
                    nc.tensor.matmul(mh_ps[:, h0:h1], vt[:, h0:h1],
                                     kr[:, h0:h1], start=(h == 0),
                                     stop=False, skip_group_check=True)
        else:
            for h in range(NHL):
                h0, h1 = h * 128, (h + 1) * 128
                nc.tensor.matmul(mh_ps[:, h0:h1], vt[:, h0:h1],
                                 kr[:, h0:h1], start=False,
                                 stop=(mg == 4 * NBLK - 1),
                                 skip_group_check=True)

    mh_pend = None
    # --- phase B: projections + rope + Mh accumulation ---
    for blk in range(NBLK):
        c0, c1 = blk * BLK, (blk + 1) * BLK
        xt = xt0 if (blk == 0 and xt0 is not None) else load_xt(blk)

        def xv(k):
            return xt[:, k * BLK:(k + 1) * BLK]

        def xl(k, m):
            return xt[:, k * BLK + m * 128:k * BLK + (m + 1) * 128]

        # Q (transposed layout, per head), rope into resident qtr
        for h in range(NHL):
            h0, h1 = h * 128, (h + 1) * 128
            q_ps = psum.tile([P, BLK], F32, name="q_ps", tag="ps")
            for k in range(KT):
                nc.tensor.matmul(q_ps[:],
                                 wq_all[:, k * DLOC + h0:k * DLOC + h1],
                                 xv(k), start=(k == 0), stop=(k == KT - 1),
                                 skip_group_check=True)
            # rope with bias folded in: out = ((q + bq) * trig)
            a = tmp.tile([P, BLK], F32, name="a", tag="a")
            nc.vector.scalar_tensor_tensor(
                a[:], q_ps[:], bqc[:, h:h + 1], c2[:, c0:c1], ADD, MULT)
            bb = tmp.tile([P, BLK], F32, name="bb", tag="bb")
            nc.vector.scalar_tensor_tensor(
                bb[0:64, :], q_ps[64:128, :], bqc[64:128, h:h + 1],
                s2x[64:128, c0:c1], ADD, MULT)
            nc.vector.scalar_tensor_tensor(
                bb[64:128, :], q_ps[0:64, :], bqc[0:64, h:h + 1],
                s2x[0:64, c0:c1], ADD, MULT)
            nc.vector.tensor_add(qtr[h][:, c0:c1], a[:], bb[:])

        # K (natural, roped), V (natural), Mh accumulation
        for m in range(4):
            mg = blk * 4 + m
            r0 = mg * 128
            k_ps = psum.tile([P, DLOC], F32, name="k_ps", tag="ps")
            nc.tensor.matmul(k_ps[:], ones[0:1, 0:128], bias["bk"][0:1, :],
                             start=True, stop=False, skip_group_check=True)
            for k in range(KT):
                nc.tensor.matmul(k_ps[:], xl(k, m),
                                 wk_all[:, k * DLOC:(k + 1) * DLOC],
                                 start=False, stop=(k == KT - 1),
                                 skip_group_check=True)
            cn = tpool.tile([P, DLOC], BF16, name="cn", tag="cn")
            nc.sync.dma_start(cn[:], d["cn4"][r0:r0 + 128, :])
            sn = tpool.tile([P, DLOC], BF16, name="sn", tag="sn")
            nc.sync.dma_start(sn[:], d["sn4x"][r0:r0 + 128, :])
            a2 = tmp.tile([P, DLOC], F32, name="a2", tag="a")
            nc.vector.tensor_mul(a2[:], k_ps[:], cn[:])
            b2 = tmp.tile([P, DLOC], F32, name="b2", tag="bb")
            nc.vector.tensor_mul(pair_view(b2)[:, :, 0, :],
                                 pair_view(k_ps)[:, :, 1, :],
                                 pair_view(sn)[:, :, 1, :])
            nc.vector.tensor_mul(pair_view(b2)[:, :, 1, :],
                                 pair_view(k_ps)[:, :, 0, :],
                                 pair_view(sn)[:, :, 0, :])
            kr = kvpool.tile([P, DLOC], BF16, name="kr", tag="kr")
            nc.vector.tensor_add(kr[:], a2[:], b2[:])

            v_ps = psum.tile([P, DLOC], F32, name="v_ps", tag="ps")
            nc.tensor.matmul(v_ps[:], ones[0:1, 0:128], bias["bv"][0:1, :],
                             start=True, stop=False, skip_group_check=True)
            for k in range(KT):
                nc.tensor.matmul(v_ps[:], xl(k, m),
                                 wv_all[:, k * DLOC:(k + 1) * DLOC],
                                 start=False, stop=(k == KT - 1),
                                 skip_group_check=True)
            vt = kvpool.tile([P, DLOC], BF16, name="vt", tag="vt")
            nc.scalar.copy(vt[:], v_ps[:])
            if dbg is not None:
                nc.sync.dma_start(dbg["kr"][r0:r0 + 128, :], kr[:])
                nc.sync.dma_start(dbg["v"][r0:r0 + 128, :], vt[:])

            # software-pipeline: Mh matmuls for the PREVIOUS m-tile, so the
            # PE stream never waits on this tile's rope DVE chain
            if mh_pend is not None:
                emit_mh(*mh_pend)
            mh_pend = (kr, vt, mg)

    emit_mh(*mh_pend)
    m_sb = kvpool.tile([P, DLOC], BF16, name="m_sb", tag="m_sb", bufs=1)
    nc.scalar.copy(m_sb[:], mh_ps[:])
    if dbg is not None:
        nc.sync.dma_start(dbg["m"][:], m_sb[:])
        for h in range(NHL):
            nc.sync.dma_start(dbg["qtr"][h * 128:(h + 1) * 128, :], qtr[h][:])

    # --- phase D: N_h = Mh @ woT_h (tiny), then out = sum_h Q_h @ N_h ---
    n_sb = []
    for h in range(NHL):
        t = kvpool.tile([P, DIN], BF16, name=f"n_sb{h}", tag=f"n_sb{h}",
                        bufs=1)
        n_sb.append(t)
    for h in range(NHL):
        h0, h1 = h * 128, (h + 1) * 128
        for n in range(4):
            n_ps = psum.tile([P, 512], F32, name="n_ps", tag="ps")
            nc.tensor.matmul(n_ps[:], m_sb[:, h0:h1],
                             wo_all[:, h * DIN + n * 512:h * DIN + (n + 1) * 512],
                             start=True, stop=True, skip_group_check=True)
            if (h + n) % 2 == 0:
                nc.scalar.copy(n_sb[h][:, n * 512:(n + 1) * 512], n_ps[:])
            else:
                nc.vector.tensor_copy(n_sb[h][:, n * 512:(n + 1) * 512], n_ps[:])
    for blk in range(NBLK):
        c0 = blk * BLK
        for m in range(4):
            for n in range(4):
                o_ps = psum.tile([P, 512], F32, name="o_ps", tag="ps")
                for h in range(NHL):
                    nc.tensor.matmul(
                        o_ps[:], qtr[h][:, c0 + m * 128:c0 + (m + 1) * 128],
                        n_sb[h][:, n * 512:(n + 1) * 512],
                        start=(h == 0), stop=(h == NHL - 1),
                        skip_group_check=True)
                ot = osb.tile([P, 512], F32, name="ot", tag="ot")
                if (m + n) % 2 == 0:
                    nc.scalar.copy(ot[:], o_ps[:])
                else:
                    nc.vector.tensor_copy(ot[:], o_ps[:])
                nc.sync.dma_start(
                    out_d[c0 + m * 128: c0 + (m + 1) * 128,
                          n * 512:(n + 1) * 512], ot[:])


def build_nc(debug_taps=False, reps=1, loop_n=0):
    global _NC_CACHE
    if _NC_CACHE is not None and not debug_taps and reps == 1 and not loop_n:
        return _NC_CACHE
    nc = bacc.Bacc("TRN2", target_bir_lowering=False, debug=False)
    d = {
        "xT": nc.dram_tensor("xT", [DIN, S], BF16, kind="ExternalInput").ap(),
        "wqT": nc.dram_tensor("wqT", [DIN, DLOC], BF16, kind="ExternalInput").ap(),
        "wkT": nc.dram_tensor("wkT", [DIN, DLOC], BF16, kind="ExternalInput").ap(),
        "wvT": nc.dram_tensor("wvT", [DIN, DLOC], BF16, kind="ExternalInput").ap(),
        "woT": nc.dram_tensor("woT", [DLOC, DIN], BF16, kind="ExternalInput").ap(),
        "c2": nc.dram_tensor("c2", [P, S], BF16, kind="ExternalInput").ap(),
        "s2x": nc.dram_tensor("s2x", [P, S], BF16, kind="ExternalInput").ap(),
        "cn4": nc.dram_tensor("cn4", [S, DLOC], BF16, kind="ExternalInput").ap(),
        "sn4x": nc.dram_tensor("sn4x", [S, DLOC], BF16, kind="ExternalInput").ap(),
        "bqc": nc.dram_tensor("bqc", [P, NHL], BF16, kind="ExternalInput").ap(),
        "bk": nc.dram_tensor("bk", [1, DLOC], BF16, kind="ExternalInput").ap(),
        "bv": nc.dram_tensor("bv", [1, DLOC], BF16, kind="ExternalInput").ap(),
    }
    out_d = nc.dram_tensor("out", [S, DIN], F32, kind="ExternalOutput").ap()
    dbg = None
    if debug_taps:
        dbg = {
            "qtr": nc.dram_tensor("dbg_qtr", [DLOC, S], BF16, kind="ExternalOutput").ap(),
            "kr": nc.dram_tensor("dbg_kr", [S, DLOC], BF16, kind="ExternalOutput").ap(),
            "v": nc.dram_tensor("dbg_v", [S, DLOC], BF16, kind="ExternalOutput").ap(),
            "m": nc.dram_tensor("dbg_m", [P, DLOC], BF16, kind="ExternalOutput").ap(),
        }
    with tile.TileContext(nc) as tc, ExitStack() as ctx:
        _emit(nc, tc, ctx, d, out_d, dbg=dbg, reps=reps, loop_n=loop_n)
    nc.compile()
    if not debug_taps and reps == 1 and not loop_n:
        _NC_CACHE = nc
    return nc


def _pair_perm():
    # within each head: 64 even pair-elements then 64 odd
    idx = np.arange(DLOC).reshape(NHL, HD)
    return np.concatenate([idx[:, 0::2], idx[:, 1::2]], axis=1).reshape(-1)


def prep_in_maps(x, freqs_cos, freqs_sin, wq_w, wq_b, wk_w, wk_b,
                 wv_w, wv_b, wo_w, wo_b):
    x = np.asarray(x, np.float32)
    cos = np.asarray(freqs_cos, np.float32)
    sin = np.asarray(freqs_sin, np.float32)
    wq_w = np.asarray(wq_w, np.float32)
    wq_b = np.asarray(wq_b, np.float32)
    wk_w = np.asarray(wk_w, np.float32)
    wk_b = np.asarray(wk_b, np.float32)
    wv_w = np.asarray(wv_w, np.float32)
    wv_b = np.asarray(wv_b, np.float32)
    wo_w = np.asarray(wo_w, np.float32)

    cosT = np.ascontiguousarray(cos.T)          # [64, S]
    sinT = np.ascontiguousarray(sin.T)
    c2 = np.concatenate([cosT, cosT], axis=0).astype(NPBF)       # [128, S]
    s2x = np.concatenate([sinT, -sinT], axis=0).astype(NPBF)
    cn4 = np.tile(np.concatenate([cos, cos], axis=1), (1, NHL)).astype(NPBF)
    sn4x = np.tile(np.concatenate([sin, -sin], axis=1), (1, NHL)).astype(NPBF)

    perm = _pair_perm()
    sc = np.float32(1.0 / np.sqrt(HD))
    in_maps = []
    for c in range(NCORES):
        b, g = divmod(c, TP)
        sl = slice(g * DLOC, (g + 1) * DLOC)
        wq_p = (wq_w[sl][perm] * sc)
        bq_p = (wq_b[sl][perm] * sc)
        wk_p = wk_w[sl][perm]
        bk_p = wk_b[sl][perm]
        wv_p = wv_w[sl]
        bv_p = wv_b[sl]
        in_maps.append({
            "xT": np.ascontiguousarray(x[b].T).astype(NPBF),
            "wqT": np.ascontiguousarray(wq_p.T).astype(NPBF),
            "wkT": np.ascontiguousarray(wk_p.T).astype(NPBF),
            "wvT": np.ascontiguousarray(wv_p.T).astype(NPBF),
            "woT": np.ascontiguousarray(wo_w[:, sl].T).astype(NPBF),
            "c2": c2, "s2x": s2x, "cn4": cn4, "sn4x": sn4x,
            "bqc": np.ascontiguousarray(bq_p.reshape(NHL, P).T).astype(NPBF),
            "bk": bk_p[None, :].astype(NPBF),
            "bv": bv_p[None, :].astype(NPBF),
        })
    return in_maps


def assemble(results, wo_b):
    wo_b = np.asarray(wo_b, np.float32)
    out = np.zeros((B, S, DIN), np.float32)
    for c, r in enumerate(results):
        out[c // TP] += r["out"]
    out += wo_b[None, None, :]
    return out


def kernel(**inputs):
    nc = build_nc()
    in_maps = prep_in_maps(
        inputs["x"], inputs["freqs_cos"], inputs["freqs_sin"],
        inputs["wq_w"], inputs["wq_b"], inputs["wk_w"], inputs["wk_b"],
        inputs["wv_w"], inputs["wv_b"], inputs["wo_w"], inputs["wo_b"])
    res = run_bass_kernel_spmd(nc, in_maps, core_ids=list(range(NCORES)))
    return assemble(res.results, inputs["wo_b"])


# revision 47
# speedup vs baseline: 1.2166x; 1.2166x over previous
"""AttentionWithRotary on 8 Trainium2 cores.

Math: reference applies raw (no-softmax) attention:
    out = ((rope(xWq^T+bq) @ rope(xWk^T+bk)^T)/sqrt(hd)) @ (xWv^T+bv) @ Wo^T + bo
Since there is no softmax, associativity gives per head:
    out_h = Q_r @ (K_r^T @ V) / sqrt(hd)
which turns the S x S score matrix into a hd x hd one.

Sharding: data-parallel on batch (2) x tensor-parallel on heads (4 heads/core).
Each core computes a row-parallel partial of the output projection; partials
are summed on the host (the "all-reduce" of row-parallel TP) and bo added.

Per-core layouts (prepped on host, all bf16):
  xT   [2048,2048]  x[b]^T  (din on partitions)
  wqT/wkT/wvT [2048,512]  weight slices transposed; q/k rows pair-split
                 permuted per head (64 even pair elems then 64 odd); q scaled
                 by 1/sqrt(hd)
  woT  [512,2048]  wo column slice transposed
  c2   [128,2048]  [cosT; cosT]        (cosT = freqs_cos^T, [64,2048])
  s2x  [128,2048]  [sinT; -sinT]
  cn4  [2048,512]  per-head [cos,cos] tiling (natural layout)
  sn4x [2048,512]  per-head [sin,-sin] tiling
  bq/bk/bv [1,512] bias slices (q/k permuted, q scaled)

On-device per core:
  QT (rope) [512,2048] transposed layout; K_r,V natural [2048,512] streamed;
  Mh[h] = K_r^T V accumulated in one PSUM bank across all seq tiles;
  attn^T[h] = Mh[h]^T @ QT[h]; partial = attn @ woT accumulated over heads.
"""
import numpy as np
import ml_dtypes
from contextlib import ExitStack

import concourse.bacc as bacc
import concourse.tile as tile
import concourse.mybir as mybir
from concourse.bass_utils import run_bass_kernel_spmd

BF16 = mybir.dt.bfloat16
F32 = mybir.dt.float32
NPBF = ml_dtypes.bfloat16

S = 2048
DIN = 2048
NH = 16
HD = 128
B = 2
NCORES = 8
TP = 4                 # head-parallel ways
NHL = NH // TP         # 4 heads per core
DLOC = NHL * HD        # 512 local head dims
BLK = 512
NBLK = S // BLK        # 4 seq blocks
KT = DIN // 128        # 16 contraction tiles
P = 128

_NC_CACHE = None


def _emit(nc, tc, ctx, d, out_d, dbg=None, reps=1, loop_n=0):
    wpool = ctx.enter_context(tc.tile_pool(name="w", bufs=1))
    xpool = ctx.enter_context(tc.tile_pool(name="x", bufs=2))
    tpool = ctx.enter_context(tc.tile_pool(name="trig", bufs=3))
    qpool = ctx.enter_context(tc.tile_pool(name="qtr", bufs=1))
    kvpool = ctx.enter_context(tc.tile_pool(name="kv", bufs=3))
    tmp = ctx.enter_context(tc.tile_pool(name="tmp", bufs=3))
    osb = ctx.enter_context(tc.tile_pool(name="osb", bufs=6))
    psum = ctx.enter_context(tc.tile_pool(name="ps", bufs=7, space="PSUM"))
    mps = ctx.enter_context(tc.tile_pool(name="mps", bufs=1, space="PSUM"))

    def load_xt(blk):
        # per-k 2D DMAs (contiguous free dim) into one wide tile; 3D
        # k-batched APs compile but are ~30x slower on real hardware
        t = xpool.tile([P, KT * BLK], BF16, name="xt", tag="xt")
        for k in range(KT):
            nc.gpsimd.dma_start(
                t[:, k * BLK:(k + 1) * BLK],
                d["xT"][k * 128:(k + 1) * 128, blk * BLK:(blk + 1) * BLK])
        return t

    # --- resident constants, in consumption order so the DMA queue serves
    # the first matmuls quickly ---
    ones = wpool.tile([1, BLK], BF16, name="ones", tag="ones")
    nc.vector.memset(ones[:], 1.0)
    bias = {}
    for nm in ("bk",):
        t = wpool.tile([1, DLOC], BF16, name=nm, tag=nm)
        nc.sync.dma_start(t[:], d[nm][:])
        bias[nm] = t
    bv_rep = wpool.tile([P, DLOC], BF16, name="bv_rep", tag="bv_rep")
    nc.sync.dma_start(bv_rep[:], d["bv_rep"][:])
    bias["bv_rep"] = bv_rep
    bqc = wpool.tile([P, NHL], BF16, name="bqc", tag="bqc")
    nc.sync.dma_start(bqc[:], d["bqc"][:])

    xt0 = load_xt(0)
    wq_all = wpool.tile([P, KT * DLOC], BF16, name="wq_all", tag="wq_all")
    for k in range(KT):
        nc.sync.dma_start(wq_all[:, k * DLOC:(k + 1) * DLOC],
                          d["wqT"][k * 128:(k + 1) * 128, :])
    c2 = wpool.tile([P, S], BF16, name="c2", tag="c2")
    nc.sync.dma_start(c2[:], d["c2"][:])
    s2x = wpool.tile([P, S], BF16, name="s2x", tag="s2x")
    nc.sync.dma_start(s2x[:], d["s2x"][:])
    wk_all = wpool.tile([P, KT * DLOC], BF16, name="wk_all", tag="wk_all")
    wv_all = wpool.tile([P, KT * DLOC], BF16, name="wv_all", tag="wv_all")
    for k in range(KT):
        nc.sync.dma_start(wk_all[:, k * DLOC:(k + 1) * DLOC],
                          d["wkT"][k * 128:(k + 1) * 128, :])
        nc.sync.dma_start(wv_all[:, k * DLOC:(k + 1) * DLOC],
                          d["wvT"][k * 128:(k + 1) * 128, :])
    wo_all = wpool.tile([P, NHL * DIN], BF16, name="wo_all", tag="wo_all")
    for h in range(NHL):
        nc.sync.dma_start(wo_all[:, h * DIN:(h + 1) * DIN],
                          d["woT"][h * 128:(h + 1) * 128, :])

    qtr = []
    for h in range(NHL):
        t = qpool.tile([P, S], BF16, name=f"qtr{h}", tag=f"qtr{h}")
        qtr.append(t)

    def pair_view(t):
        return t.rearrange("p (h u j) -> p h u j", h=NHL, u=2, j=64)

    if loop_n:
        with tc.For_i(0, loop_n, 1) as _i:
            _emit_compute(nc, tc, d, out_d, dbg, tpool, kvpool, tmp,
                          osb, psum, mps, ones, bias, bqc, c2, s2x, wq_all,
                          wk_all, wv_all, wo_all, qtr, pair_view, load_xt,
                          None)
    else:
        for _rep in range(reps):
            _emit_compute(nc, tc, d, out_d, dbg, tpool, kvpool, tmp,
                          osb, psum, mps, ones, bias, bqc, c2, s2x, wq_all,
                          wk_all, wv_all, wo_all, qtr, pair_view, load_xt,
                          xt0 if _rep == 0 else None)


def _emit_compute(nc, tc, d, out_d, dbg, tpool, kvpool, tmp, osb, psum,
                  mps, ones, bias, bqc, c2, s2x, wq_all, wk_all, wv_all,
                  wo_all, qtr, pair_view, load_xt, xt0):
    ADD = mybir.AluOpType.add
    MULT = mybir.AluOpType.mult
    mh_ps = mps.tile([P, DLOC], F32, name="mh_ps", tag="mh")

    def emit_mh(kr, vt, mg):
        if mg == 0:
            # start=True clears the whole PSUM bank, so only the first
            # matmul may carry it; the critical section pins the order
            # of the four first-writes within the shared bank.
            with tc.tile_critical():
                for h in range(NHL):
                    h0, h1 = h * 128, (h + 1) * 128
                    nc.tensor.matmul(mh_ps[:, h0:h1], vt[:, h0:h1],
                                     kr[:, h0:h1], start=(h == 0),
                                     stop=False, skip_group_check=True)
        else:
            for h in range(NHL):
                h0, h1 = h * 128, (h + 1) * 128
                nc.tensor.matmul(mh_ps[:, h0:h1], vt[:, h0:h1],
                                 kr[:, h0:h1], start=False,
                                 stop=(mg == 4 * NBLK - 1),
                                 skip_group_check=True)

    mh_pend = None
    # --- phase B: projections + rope + Mh accumulation ---
    for blk in range(NBLK):
        c0, c1 = blk * BLK, (blk + 1) * BLK
        xt = xt0 if (blk == 0 and xt0 is not None) else load_xt(blk)

        def xv(k):
            return xt[:, k * BLK:(k + 1) * BLK]

        def xl(k, m):
            return xt[:, k * BLK + m * 128:k * BLK + (m + 1) * 128]

        # Q (transposed layout, per head), rope into resident qtr
        for h in range(NHL):
            h0, h1 = h * 128, (h + 1) * 128
            q_ps = psum.tile([P, BLK], F32, name="q_ps", tag="ps")
            for k in range(KT):
                nc.tensor.matmul(q_ps[:],
                                 wq_all[:, k * DLOC + h0:k * DLOC + h1],
                                 xv(k), start=(k == 0), stop=(k == KT - 1),
                                 skip_group_check=True)
            # rope with bias folded in: out = ((q + bq) * trig)
            a = tmp.tile([P, BLK], F32, name="a", tag="a")
            nc.vector.scalar_tensor_tensor(
                a[:], q_ps[:], bqc[:, h:h + 1], c2[:, c0:c1], ADD, MULT)
            bb = tmp.tile([P, BLK], F32, name="bb", tag="bb")
            nc.vector.scalar_tensor_tensor(
                bb[0:64, :], q_ps[64:128, :], bqc[64:128, h:h + 1],
                s2x[64:128, c0:c1], ADD, MULT)
            nc.vector.scalar_tensor_tensor(
                bb[64:128, :], q_ps[0:64, :], bqc[0:64, h:h + 1],
                s2x[0:64, c0:c1], ADD, MULT)
            nc.vector.tensor_add(qtr[h][:, c0:c1], a[:], bb[:])

        # K (natural, roped), V (natural), Mh accumulation
        for m in range(4):
            mg = blk * 4 + m
            r0 = mg * 128
            k_ps = psum.tile([P, DLOC], F32, name="k_ps", tag="ps")
            nc.tensor.matmul(k_ps[:], ones[0:1, 0:128], bias["bk"][0:1, :],
                             start=True, stop=False, skip_group_check=True)
            for k in range(KT):
                nc.tensor.matmul(k_ps[:], xl(k, m),
                                 wk_all[:, k * DLOC:(k + 1) * DLOC],
                                 start=False, stop=(k == KT - 1),
                                 skip_group_check=True)
            cn = tpool.tile([P, DLOC], BF16, name="cn", tag="cn")
            nc.sync.dma_start(cn[:], d["cn4"][r0:r0 + 128, :])
            sn = tpool.tile([P, DLOC], BF16, name="sn", tag="sn")
            nc.sync.dma_start(sn[:], d["sn4x"][r0:r0 + 128, :])
            a2 = tmp.tile([P, DLOC], F32, name="a2", tag="a")
            nc.vector.tensor_mul(a2[:], k_ps[:], cn[:])
            b2 = tmp.tile([P, DLOC], F32, name="b2", tag="bb")
            nc.vector.tensor_mul(pair_view(b2)[:, :, 0, :],
                                 pair_view(k_ps)[:, :, 1, :],
                                 pair_view(sn)[:, :, 1, :])
            nc.vector.tensor_mul(pair_view(b2)[:, :, 1, :],
                                 pair_view(k_ps)[:, :, 0, :],
                                 pair_view(sn)[:, :, 0, :])
            kr = kvpool.tile([P, DLOC], BF16, name="kr", tag="kr")
            nc.vector.tensor_add(kr[:], a2[:], b2[:])

            v_ps = psum.tile([P, DLOC], F32, name="v_ps", tag="ps")
            for k in range(KT):
                nc.tensor.matmul(v_ps[:], xl(k, m),
                                 wv_all[:, k * DLOC:(k + 1) * DLOC],
                                 start=(k == 0), stop=(k == KT - 1),
                                 skip_group_check=True)
            vt = kvpool.tile([P, DLOC], BF16, name="vt", tag="vt")
            nc.vector.tensor_add(vt[:], v_ps[:], bias["bv_rep"][:])
            if dbg is not None:
                nc.sync.dma_start(dbg["kr"][r0:r0 + 128, :], kr[:])
                nc.sync.dma_start(dbg["v"][r0:r0 + 128, :], vt[:])

            # software-pipeline: Mh matmuls for the PREVIOUS m-tile, so the
            # PE stream never waits on this tile's rope DVE chain
            if mh_pend is not None:
                emit_mh(*mh_pend)
            mh_pend = (kr, vt, mg)

    emit_mh(*mh_pend)
    m_sb = kvpool.tile([P, DLOC], BF16, name="m_sb", tag="m_sb", bufs=1)
    nc.scalar.copy(m_sb[:], mh_ps[:])
    if dbg is not None:
        nc.sync.dma_start(dbg["m"][:], m_sb[:])
        for h in range(NHL):
            nc.sync.dma_start(dbg["qtr"][h * 128:(h + 1) * 128, :], qtr[h][:])

    # --- phase D: N_h = Mh @ woT_h (tiny), then out = sum_h Q_h @ N_h ---
    n_sb = []
    for h in range(NHL):
        t = kvpool.tile([P, DIN], BF16, name=f"n_sb{h}", tag=f"n_sb{h}",
                        bufs=1)
        n_sb.append(t)
    for h in range(NHL):
        h0, h1 = h * 128, (h + 1) * 128
        for n in range(4):
            n_ps = psum.tile([P, 512], F32, name="n_ps", tag="ps")
            nc.tensor.matmul(n_ps[:], m_sb[:, h0:h1],
                             wo_all[:, h * DIN + n * 512:h * DIN + (n + 1) * 512],
                             start=True, stop=True, skip_group_check=True)
            if (h + n) % 2 == 0:
                nc.scalar.copy(n_sb[h][:, n * 512:(n + 1) * 512], n_ps[:])
            else:
                nc.vector.tensor_copy(n_sb[h][:, n * 512:(n + 1) * 512], n_ps[:])
    for blk in range(NBLK):
        c0 = blk * BLK
        for m in range(4):
            for n in range(4):
                o_ps = psum.tile([P, 512], F32, name="o_ps", tag="ps")
                for h in range(NHL):
                    nc.tensor.matmul(
                        o_ps[:], qtr[h][:, c0 + m * 128:c0 + (m + 1) * 128],
                        n_sb[h][:, n * 512:(n + 1) * 512],
                        start=(h == 0), stop=(h == NHL - 1),
                        skip_group_check=True)
                ot = osb.tile([P, 512], F32, name="ot", tag="ot")
                if (m + n) % 2 == 0:
                    nc.scalar.copy(ot[:], o_ps[:])
                else:
                    nc.vector.tensor_copy(ot[:], o_ps[:])
                nc.sync.dma_start(
                    out_d[c0 + m * 128: c0 + (m + 1) * 128,
                          n * 512:(n + 1) * 512], ot[:])


def build_nc(debug_taps=False, reps=1, loop_n=0):
    global _NC_CACHE
    if _NC_CACHE is not None and not debug_taps and reps == 1 and not loop_n:
        return _NC_CACHE
    nc = bacc.Bacc("TRN2", target_bir_lowering=False, debug=False)
    d = {
        "xT": nc.dram_tensor("xT", [DIN, S], BF16, kind="ExternalInput").ap(),
        "wqT": nc.dram_tensor("wqT", [DIN, DLOC], BF16, kind="ExternalInput").ap(),
        "wkT": nc.dram_tensor("wkT", [DIN, DLOC], BF16, kind="ExternalInput").ap(),
        "wvT": nc.dram_tensor("wvT", [DIN, DLOC], BF16, kind="ExternalInput").ap(),
        "woT": nc.dram_tensor("woT", [DLOC, DIN], BF16, kind="ExternalInput").ap(),
        "c2": nc.dram_tensor("c2", [P, S], BF16, kind="ExternalInput").ap(),
        "s2x": nc.dram_tensor("s2x", [P, S], BF16, kind="ExternalInput").ap(),
        "cn4": nc.dram_tensor("cn4", [S, DLOC], BF16, kind="ExternalInput").ap(),
        "sn4x": nc.dram_tensor("sn4x", [S, DLOC], BF16, kind="ExternalInput").ap(),
        "bqc": nc.dram_tensor("bqc", [P, NHL], BF16, kind="ExternalInput").ap(),
        "bk": nc.dram_tensor("bk", [1, DLOC], BF16, kind="ExternalInput").ap(),
        "bv_rep": nc.dram_tensor("bv_rep", [P, DLOC], BF16, kind="ExternalInput").ap(),
    }
    out_d = nc.dram_tensor("out", [S, DIN], F32, kind="ExternalOutput").ap()
    dbg = None
    if debug_taps:
        dbg = {
            "qtr": nc.dram_tensor("dbg_qtr", [DLOC, S], BF16, kind="ExternalOutput").ap(),
            "kr": nc.dram_tensor("dbg_kr", [S, DLOC], BF16, kind="ExternalOutput").ap(),
            "v": nc.dram_tensor("dbg_v", [S, DLOC], BF16, kind="ExternalOutput").ap(),
            "m": nc.dram_tensor("dbg_m", [P, DLOC], BF16, kind="ExternalOutput").ap(),
        }
    with tile.TileContext(nc) as tc, ExitStack() as ctx:
        _emit(nc, tc, ctx, d, out_d, dbg=dbg, reps=reps, loop_n=loop_n)
    nc.compile()
    if not debug_taps and reps == 1 and not loop_n:
        _NC_CACHE = nc
    return nc


def _pair_perm():
    # within each head: 64 even pair-elements then 64 odd
    idx = np.arange(DLOC).reshape(NHL, HD)
    return np.concatenate([idx[:, 0::2], idx[:, 1::2]], axis=1).reshape(-1)


def prep_in_maps(x, freqs_cos, freqs_sin, wq_w, wq_b, wk_w, wk_b,
                 wv_w, wv_b, wo_w, wo_b):
    x = np.asarray(x, np.float32)
    cos = np.asarray(freqs_cos, np.float32)
    sin = np.asarray(freqs_sin, np.float32)
    wq_w = np.asarray(wq_w, np.float32)
    wq_b = np.asarray(wq_b, np.float32)
    wk_w = np.asarray(wk_w, np.float32)
    wk_b = np.asarray(wk_b, np.float32)
    wv_w = np.asarray(wv_w, np.float32)
    wv_b = np.asarray(wv_b, np.float32)
    wo_w = np.asarray(wo_w, np.float32)

    cosT = np.ascontiguousarray(cos.T)          # [64, S]
    sinT = np.ascontiguousarray(sin.T)
    c2 = np.concatenate([cosT, cosT], axis=0).astype(NPBF)       # [128, S]
    s2x = np.concatenate([sinT, -sinT], axis=0).astype(NPBF)
    cn4 = np.tile(np.concatenate([cos, cos], axis=1), (1, NHL)).astype(NPBF)
    sn4x = np.tile(np.concatenate([sin, -sin], axis=1), (1, NHL)).astype(NPBF)

    perm = _pair_perm()
    sc = np.float32(1.0 / np.sqrt(HD))
    in_maps = []
    for c in range(NCORES):
        b, g = divmod(c, TP)
        sl = slice(g * DLOC, (g + 1) * DLOC)
        wq_p = (wq_w[sl][perm] * sc)
        bq_p = (wq_b[sl][perm] * sc)
        wk_p = wk_w[sl][perm]
        bk_p = wk_b[sl][perm]
        wv_p = wv_w[sl]
        bv_p = wv_b[sl]
        in_maps.append({
            "xT": np.ascontiguousarray(x[b].T).astype(NPBF),
            "wqT": np.ascontiguousarray(wq_p.T).astype(NPBF),
            "wkT": np.ascontiguousarray(wk_p.T).astype(NPBF),
            "wvT": np.ascontiguousarray(wv_p.T).astype(NPBF),
            "woT": np.ascontiguousarray(wo_w[:, sl].T).astype(NPBF),
            "c2": c2, "s2x": s2x, "cn4": cn4, "sn4x": sn4x,
            "bqc": np.ascontiguousarray(bq_p.reshape(NHL, P).T).astype(NPBF),
            "bk": bk_p[None, :].astype(NPBF),
            "bv_rep": np.broadcast_to(bv_p[None, :], (P, DLOC)).astype(NPBF),
        })
    return in_maps


def assemble(results, wo_b):
    wo_b = np.asarray(wo_b, np.float32)
    out = np.zeros((B, S, DIN), np.float32)
    for c, r in enumerate(results):
        out[c // TP] += r["out"]
    out += wo_b[None, None, :]
    return out


def kernel(**inputs):
    nc = build_nc()
    in_maps = prep_in_maps(
        inputs["x"], inputs["freqs_cos"], inputs["freqs_sin"],
        inputs["wq_w"], inputs["wq_b"], inputs["wk_w"], inputs["wk_b"],
        inputs["wv_w"], inputs["wv_b"], inputs["wo_w"], inputs["wo_b"])
    res = run_bass_kernel_spmd(nc, in_maps, core_ids=list(range(NCORES)))
    return assemble(res.results, inputs["wo_b"])
